# revision 11
# baseline (speedup 1.0000x reference)
"""DigitCaps routing kernel for 8x TRN2 NeuronCores.

Problem: x (256,1152,32) f32, W (1152,10,32,16) f32 ->
  u_hat = einsum('brc,rnco->brno', x, W); 3 rounds of dynamic routing -> v (256,10,16).

Strategy (data-parallel over batch, 32 b's per core):
  - Host pre-transposes x and W into PE-friendly layouts (bf16).
  - Einsum on PE with 16-way tile_position packing: per group of 16 r's
    (r = 16G+4i+j), tile (i,j) computes u_hat[b, r, :, :] = x_r^T.T @ W_r
    with K=c(32) on partition strip i, output partition strip j.
  - u_hat kept resident in SBUF as bf16, layout [p=32j+b][G, i, n, o].
  - Routing on-chip: softmax/elementwise on DVE/ACT; the r-sum for
    s = sum_r c*u_hat runs on PE via a constant 4-stacked-identity lhsT
    (sums the j partition strips) with PSUM accumulation over (G,i).
  - agreement = sum_o u_hat*v via DVE mul + halving adds (o: 16->1).
"""

import numpy as np
import ml_dtypes

BF16 = ml_dtypes.bfloat16

B, R, N, C, O = 256, 1152, 10, 32, 16
NCORES = 8
BS = B // NCORES          # 32 batch per core
NG = R // 16              # 72 groups of 16 r's; r = 16G + 4i + j
NO = N * O                # 160
GC = 8                    # G-chunk for routing elementwise ops (72 = 9*8)
NCH = NG // GC            # 9 chunks
EPS = 1e-8

_CACHE = {}


def _build_program(niter=3, dump=()):
    import concourse.bacc as bacc
    import concourse.tile as tile
    import concourse.mybir as mybir

    f32 = mybir.dt.float32
    bf16 = mybir.dt.bfloat16
    alu = mybir.AluOpType
    act = mybir.ActivationFunctionType

    nc = bacc.Bacc("TRN2", target_bir_lowering=False, debug=False)

    # xt[32i+c][G, j, b] = x[b, 16G+4i+j, c]
    xt_d = nc.dram_tensor("xt", [128, NG, 4, BS], bf16, kind="ExternalInput")
    # wt[G][32i+c][j][n*16+o] = W[16G+4i+j, n, c, o]
    wt_d = nc.dram_tensor("wt", [NG, 128, 4, NO], bf16, kind="ExternalInput")
    # i4[32j+b][b'] = (b == b')
    i4_d = nc.dram_tensor("i4", [128, BS], bf16, kind="ExternalInput")
    v_d = nc.dram_tensor("v", [BS, NO], f32, kind="ExternalOutput")
    dump_d = {}
    if "u" in dump:
        dump_d["u"] = nc.dram_tensor("du", [128, 2, 4, N, O], bf16, kind="ExternalOutput")
    if "s" in dump:
        dump_d["s"] = nc.dram_tensor("ds", [BS, N, O], f32, kind="ExternalOutput")
    if "bl" in dump:
        dump_d["bl"] = nc.dram_tensor("dbl", [128, NG, 4, N], f32, kind="ExternalOutput")
    if "vb" in dump:
        dump_d["vb"] = nc.dram_tensor("dvb", [128, N, O], bf16, kind="ExternalOutput")
    if "p2" in dump:
        dump_d["p2"] = nc.dram_tensor("dp2", [128, 2, 4, N, O], bf16, kind="ExternalOutput")

    with tile.TileContext(nc) as tc:
        from contextlib import ExitStack

        with ExitStack() as ctx:
            persist = ctx.enter_context(tc.tile_pool(name="persist", bufs=1))
            u_sb = persist.tile([128, NG, 4, N, O], bf16, name="u_sb")
            bl_sb = persist.tile([128, NG, 4, N], f32, name="bl_sb")
            i4_sb = persist.tile([128, BS], bf16, name="i4_sb")
            nc.sync.dma_start(i4_sb[:], i4_d[:])

            # ---------------- Phase 1: einsum  ----------------
            with ExitStack() as pctx:
                xt_pool = pctx.enter_context(tc.tile_pool(name="xt", bufs=1))
                wt_pool = pctx.enter_context(tc.tile_pool(name="wt", bufs=3))
                ps_pool = pctx.enter_context(
                    tc.tile_pool(name="ps", bufs=2, space="PSUM")
                )

                xt_sb = xt_pool.tile([128, NG, 4, BS], bf16, name="xt_sb")
                nc.sync.dma_start(xt_sb[:], xt_d[:])

                for sg in range(NG // 3):  # super-groups of 3 G's
                    psums = [
                        ps_pool.tile([128, 3, NO], f32, name=f"ps{i}", tag=f"ps{i}")
                        for i in range(4)
                    ]
                    for slot in range(3):
                        g = sg * 3 + slot
                        wt_t = wt_pool.tile([128, 4, NO], bf16, name="wt_t", tag="wt")
                        nc.sync.dma_start(wt_t[:], wt_d[g])
                        for i in range(4):
                            for j in range(4):
                                nc.tensor.matmul(
                                    psums[i][32 * j : 32 * (j + 1), slot, :],
                                    xt_sb[32 * i : 32 * (i + 1), g, j, :],
                                    wt_t[32 * i : 32 * (i + 1), j, :],
                                    start=True,
                                    stop=True,
                                    tile_position=(32 * i, 32 * j),
                                )
                    # copy 3 G's worth per i-strip into u_sb (cast to bf16)
                    for i in range(4):
                        nc.scalar.activation(
                            u_sb[:, sg * 3 : sg * 3 + 3, i, :, :],
                            psums[i][:, :, :],
                            act.Copy,
                        )

            if "u" in dump:
                nc.sync.dma_start(dump_d["u"][:], u_sb[:, 0:2, :, :, :])

            # ---------------- Phase 2: routing ----------------
            with ExitStack() as rctx:
                rpool = rctx.enter_context(tc.tile_pool(name="rt", bufs=1))
                ch_pool = rctx.enter_context(tc.tile_pool(name="ch", bufs=2))
                sps_pool = rctx.enter_context(
                    tc.tile_pool(name="sps", bufs=2, space="PSUM")
                )

                for t in range(3):
                    # ---- c coefficients ----
                    if t > 0:
                        e_sb = rpool.tile([128, NG, 4, N], f32, name="e_sb", tag="e")
                        nc.scalar.activation(e_sb[:], bl_sb[:], act.Exp)
                        den = rpool.tile([128, NG, 4], f32, name="den", tag="den")
                        nc.vector.tensor_reduce(
                            den[:], e_sb[:], axis=mybir.AxisListType.X, op=alu.add
                        )
                        rec = rpool.tile([128, NG, 4], f32, name="rec", tag="rec")
                        nc.vector.reciprocal(rec[:], den[:])
                        c_sb = rpool.tile([128, NG, 4, N], bf16, name="c_sb", tag="c")
                        rec_b = rec[:].rearrange("p g i -> p g i ()").broadcast_to(
                            [128, NG, 4, N]
                        )
                        nc.vector.tensor_mul(c_sb[:], e_sb[:], rec_b)

                    # ---- s = sum_r c * u_hat  (PE accumulates over G,i and j) ----
                    s_ps = sps_pool.tile([BS, N, O], f32, name="s_ps", tag="s_ps")
                    n_mm = NCH * GC * 4
                    mm_k = 0
                    for ch in range(NCH):
                        u_chunk = u_sb[:, ch * GC : (ch + 1) * GC, :, :, :]
                        prod = ch_pool.tile(
                            [128, GC, 4, N, O], bf16, name="prod", tag="prod"
                        )
                        if t == 0:
                            nc.vector.tensor_scalar_mul(prod[:], u_chunk, 0.1)
                        else:
                            c16 = ch_pool.tile(
                                [128, GC, 4, N, O], bf16, name="c16", tag="c16"
                            )
                            c_b = (
                                c_sb[:, ch * GC : (ch + 1) * GC, :, :]
                                .rearrange("p g i n -> p g i n ()")
                                .broadcast_to([128, GC, 4, N, O])
                            )
                            nc.scalar.activation(c16[:], c_b, act.Copy)
                            nc.vector.tensor_mul(prod[:], u_chunk, c16[:])
                        for g in range(GC):
                            for i in range(4):
                                nc.tensor.matmul(
                                    s_ps[:, :, :],
                                    i4_sb[:],
                                    prod[:, g, i, :, :],
                                    start=(mm_k == 0),
                                    stop=(mm_k == n_mm - 1),
                                    skip_group_check=True,
                                )
                                mm_k += 1

                    if t == 0 and "s" in dump:
                        s_dbg = rpool.tile([BS, N, O], f32, name="s_dbg", tag="s_dbg")
                        nc.scalar.activation(s_dbg[:], s_ps[:], act.Copy)
                        nc.sync.dma_start(dump_d["s"][:], s_dbg[:])

                    # ---- squash ----
                    sq = rpool.tile([BS, N, O], f32, name="sq", tag="sq")
                    nc.scalar.activation(sq[:], s_ps[:], act.Square)
                    ssum = rpool.tile([BS, N], f32, name="ssum", tag="ssum")
                    nc.vector.tensor_reduce(
                        ssum[:], sq[:], axis=mybir.AxisListType.X, op=alu.add
                    )
                    d1 = rpool.tile([BS, N], f32, name="d1", tag="d1")
                    nc.vector.tensor_scalar_add(d1[:], ssum[:], 1.0)
                    se = rpool.tile([BS, N], f32, name="se", tag="se")
                    nc.vector.tensor_scalar_add(se[:], ssum[:], EPS)
                    sr = rpool.tile([BS, N], f32, name="sr", tag="sr")
                    nc.scalar.activation(sr[:], se[:], act.Sqrt)
                    den2 = rpool.tile([BS, N], f32, name="den2", tag="den2")
                    nc.vector.tensor_mul(den2[:], d1[:], sr[:])
                    rden = rpool.tile([BS, N], f32, name="rden", tag="rden")
                    nc.vector.reciprocal(rden[:], den2[:])
                    scale = rpool.tile([BS, N], f32, name="scale", tag="scale")
                    nc.vector.tensor_mul(scale[:], ssum[:], rden[:])
                    v_sb = rpool.tile([BS, N, O], f32, name="v_sb", tag="v_sb")
                    scale_b = scale[:].rearrange("p n -> p n ()").broadcast_to(
                        [BS, N, O]
                    )
                    nc.vector.tensor_mul(v_sb[:], s_ps[:], scale_b)

                    if t == niter - 1:
                        nc.sync.dma_start(v_d[:], v_sb[:].rearrange("p n o -> p (n o)"))
                        break

                    # ---- vb: v cast to bf16, replicated on all 4 j-strips ----
                    vb = rpool.tile([128, N, O], bf16, name="vb", tag="vb")
                    nc.vector.tensor_copy(vb[0:BS, :, :], v_sb[:])
                    for jj in range(1, 4):
                        nc.sync.dma_start(
                            vb[32 * jj : 32 * (jj + 1), :, :], vb[0:BS, :, :]
                        )

                    if t == 0 and "vb" in dump:
                        nc.sync.dma_start(dump_d["vb"][:], vb[:])

                    # ---- agreement: bl (+)= sum_o u_hat * v ----
                    for ch in range(NCH):
                        u_chunk = u_sb[:, ch * GC : (ch + 1) * GC, :, :, :]
                        vb_b = (
                            vb[:]
                            .rearrange("p n o -> p () () n o")
                            .broadcast_to([128, GC, 4, N, O])
                        )
                        p2 = ch_pool.tile(
                            [128, GC, 4, N, O], bf16, name="p2", tag="prod"
                        )
                        nc.vector.tensor_mul(p2[:], u_chunk, vb_b)
                        if t == 0 and ch == 0 and "p2" in dump:
                            nc.sync.dma_start(dump_d["p2"][:], p2[:, 0:2])
                        h1 = ch_pool.tile([128, GC, 4, N, 8], bf16, name="h1", tag="h1")
                        nc.vector.tensor_add(
                            h1[:], p2[:, :, :, :, 0:8], p2[:, :, :, :, 8:16]
                        )
                        h2 = ch_pool.tile([128, GC, 4, N, 4], bf16, name="h2", tag="h2")
                        nc.vector.tensor_add(
                            h2[:], h1[:, :, :, :, 0:4], h1[:, :, :, :, 4:8]
                        )
                        h3 = ch_pool.tile([128, GC, 4, N, 2], bf16, name="h3", tag="h3")
                        nc.vector.tensor_add(
                            h3[:], h2[:, :, :, :, 0:2], h2[:, :, :, :, 2:4]
                        )
                        bl_slice = bl_sb[:, ch * GC : (ch + 1) * GC, :, :]
                        if t == 0:
                            nc.vector.tensor_add(
                                bl_slice.rearrange("p g i n -> p g i n ()"),
                                h3[:, :, :, :, 0:1],
                                h3[:, :, :, :, 1:2],
                            )
                        else:
                            agr = ch_pool.tile(
                                [128, GC, 4, N], f32, name="agr", tag="agr"
                            )
                            nc.vector.tensor_add(
                                agr[:].rearrange("p g i n -> p g i n ()"),
                                h3[:, :, :, :, 0:1],
                                h3[:, :, :, :, 1:2],
                            )
                            nc.vector.tensor_add(bl_slice, bl_slice, agr[:])
                    if t == 0 and "bl" in dump:
                        nc.sync.dma_start(dump_d["bl"][:], bl_sb[:])

    nc.compile()
    return nc


def _prep_inputs(x, W):
    # xt[core][32i+c][G, j, b] = x[b0+b, 16G+4i+j, c]
    xr = x.reshape(NCORES, BS, NG, 4, 4, C)  # k, b, G, i, j, c
    xt = np.ascontiguousarray(
        xr.transpose(0, 3, 5, 2, 4, 1), dtype=BF16
    ).reshape(NCORES, 128, NG, 4, BS)
    # wt[G][32i+c][j][n*16+o] = W[16G+4i+j, n, c, o]
    wr = W.reshape(NG, 4, 4, N, C, O)  # G, i, j, n, c, o
    wt = np.ascontiguousarray(
        wr.transpose(0, 1, 4, 2, 3, 5), dtype=BF16
    ).reshape(NG, 128, 4, NO)
    i4 = np.ascontiguousarray(np.tile(np.eye(BS), (4, 1)), dtype=BF16)
    return xt, wt, i4


def kernel(x: np.ndarray, W: np.ndarray) -> np.ndarray:
    from concourse import bass_utils

    if "nc" not in _CACHE:
        _CACHE["nc"] = _build_program()
    nc = _CACHE["nc"]

    xt, wt, i4 = _prep_inputs(np.asarray(x, np.float32), np.asarray(W, np.float32))
    in_maps = [
        {"xt": np.ascontiguousarray(xt[k]), "wt": wt, "i4": i4}
        for k in range(NCORES)
    ]
    import os
    trace = bool(int(os.environ.get("KERNEL_TRACE", "0")))
    res = bass_utils.run_bass_kernel_spmd(
        nc, in_maps, core_ids=list(range(NCORES)), trace=trace
    )
    if trace:
        _CACHE["last_results"] = res
        print(f"HW exec time: {res.exec_time_ns} ns")
        print(f"trace: {res.instructions_and_trace[1] if res.instructions_and_trace else None}")
        print(f"profile_json: {res.profile_json}")
    out = np.concatenate(
        [res.results[k]["v"].reshape(BS, N, O) for k in range(NCORES)], axis=0
    )
    return out.astype(np.float32)


if __name__ == "__main__":
    x = np.random.randn(B, R, C).astype(np.float32)
    W = (np.random.randn(R, N, C, O) * 0.01).astype(np.float32)
    v = kernel(x, W)
    print("out", v.shape, v.dtype, float(np.abs(v).max()))
